# revision 1
# baseline (speedup 1.0000x reference)
"""Trainium2 Bass kernel for nn_CSPVLayer (GNN message passing), 8 NeuronCores.

Strategy: partition NODES across cores (6272/core, padded N=50176). Host sorts
edges by src node and assigns each edge to the core owning its src; scatter-mean
is then fully core-local (no collectives). Per 128-node block, edges are padded
to 128-multiples, split into lo/hi dst-index windows (dma_gather idx is int16).
Edge MLP runs feature-on-partition (W-stationary); h[dst]/v[dst] rows arrive
pre-transposed via dma_gather(transpose=True) from an fp16 [h|v] table; h[src]
contributions come from a per-block indicator matmul (no src gather). Scatter is
an indicator.T @ ef matmul accumulating sums+counts in PSUM.
"""
import math
import numpy as np

N, E0, G, H, D = 50000, 400000, 256, 128, 128
NCORES = 8
NPAD = 50176            # 392 blocks of 128
NPC = NPAD // NCORES    # 6272 nodes/core
NBLK = NPC // 128       # 49 blocks/core
TBL = 32768             # gather-window rows (int16 idx limit)
WIN = NPAD - TBL        # hi window base = 17408
SPLIT = 25088           # dst < SPLIT -> lo window
CHUNK = 512


def _round128(x):
    return ((x + 127) // 128) * 128


def _pack_idx(vals, T):
    """int16 idx values [T] -> [128, T//16] wrapped layout, replicated x8."""
    arr = np.zeros((128, T // 16), np.int16)
    cols = np.arange(T) // 16
    rows = np.arange(T) % 16
    for rep in range(8):
        arr[rows + 16 * rep, cols] = vals
    return arr


def kernel(**inputs):
    import concourse.bass as bass
    import concourse.bacc as bacc
    import concourse.mybir as mybir
    import concourse.tile as tile
    from concourse.bass_utils import run_bass_kernel_spmd

    f16, f32, i16 = mybir.dt.float16, mybir.dt.float32, mybir.dt.int16
    AF = mybir.ActivationFunctionType
    OP = mybir.AluOpType

    pos_diff = np.asarray(inputs["pos_diff"], np.float32)
    v = np.asarray(inputs["v"], np.float32)
    h = np.asarray(inputs["node_features"], np.float32)
    l = np.asarray(inputs["l"], np.float32)
    eni = np.asarray(inputs["edge_node_index"]).astype(np.int64)
    egi = np.asarray(inputs["edge_graph_index"]).astype(np.int64)
    E = pos_diff.shape[0]
    src, dst = eni[0], eni[1]

    # ---- weight algebra (host, exact f32 then cast) ----
    e_w1 = np.asarray(inputs["e_w1"], np.float32)
    W_hi = e_w1[0:128]
    W_hj = e_w1[128:256]
    W_l = e_w1[256:262]           # [6, H]
    W_v = e_w1[262:390]           # [128, H]
    W_pd = e_w1[390:518]
    vproj_w = np.asarray(inputs["vproj_w"], np.float32)
    vproj_b = np.asarray(inputs["vproj_b"], np.float32)
    Wv3 = vproj_w @ W_v           # [3, H]
    b1 = np.asarray(inputs["e_b1"], np.float32) + vproj_b @ W_v  # [H]
    Wlb = np.concatenate([W_l, b1[None, :]], 0)  # [7, H] (bias via l-row ones)
    e_w2 = np.asarray(inputs["e_w2"], np.float32)
    b2 = np.asarray(inputs["e_b2"], np.float32)
    n_w1 = np.asarray(inputs["n_w1"], np.float32)
    b3 = np.asarray(inputs["n_b1"], np.float32)
    n_w2 = np.asarray(inputs["n_w2"], np.float32)
    b4 = np.asarray(inputs["n_b2"], np.float32)

    # ---- tables ----
    hv = np.zeros((NPAD, 256), np.float16)
    hv[:N, 0:128] = h.astype(np.float16)
    hv[:N, 128:131] = v.astype(np.float16)
    ltab = np.zeros((G, 128), np.float16)
    ltab[:, 0:6] = l.astype(np.float16)
    ltab[:, 6] = 1.0              # ones row -> folds bias b1
    hv_lo, hv_hi = hv[0:TBL], hv[WIN:WIN + TBL]

    # ---- per-core edge partition, sort, block/seg grouping ----
    core_of = np.minimum(src // NPC, NCORES - 1)
    per_core = []  # list of dict(blk -> (lo_edges_idx, hi_edges_idx))
    for c in range(NCORES):
        sel = np.where(core_of == c)[0]
        s = sel[np.argsort(src[sel], kind="stable")]
        blk = (src[s] - c * NPC) // 128
        islo = dst[s] < SPLIT
        segs = {}
        for b in range(NBLK):
            m = blk == b
            segs[b] = (s[m & islo], s[m & ~islo])
        per_core.append(segs)

    caps = np.zeros((NBLK, 2), np.int64)
    for b in range(NBLK):
        for sgi in range(2):
            caps[b, sgi] = _round128(
                max(len(per_core[c][b][sgi]) for c in range(NCORES)))
    assert caps.sum(1).min() > 0, "empty block"
    T = int(caps.sum())

    # ---- packed per-core edge arrays ----
    idx_hv = np.zeros((NCORES, T), np.int64)
    idx_l = np.zeros((NCORES, T), np.int64)
    sl_row = np.full((NCORES, 1, T), -1.0, np.float16)
    posr = np.zeros((NCORES, 1, T), np.float32)
    for c in range(NCORES):
        off = 0
        for b in range(NBLK):
            for sgi in range(2):
                e_ids = per_core[c][b][sgi]
                n = len(e_ids)
                cp = int(caps[b, sgi])
                if n:
                    d = dst[e_ids]
                    idx_hv[c, off:off + n] = d if sgi == 0 else d - WIN
                    idx_l[c, off:off + n] = egi[e_ids]
                    sl_row[c, 0, off:off + n] = (src[e_ids] - c * NPC - b * 128
                                                 ).astype(np.float16)
                    posr[c, 0, off:off + n] = pos_diff[e_ids]
                off += cp
    sl_col = np.ascontiguousarray(
        sl_row[:, 0, :].reshape(NCORES, T // 128, 128).transpose(0, 2, 1)
    ).astype(np.float32)
    idx_hv_p = np.stack([_pack_idx(idx_hv[c], T) for c in range(NCORES)])
    idx_l_p = np.stack([_pack_idx(idx_l[c], T) for c in range(NCORES)])
    idx_own = _pack_idx(np.arange(NPC), NPC)

    # per-core own tables
    hv_own = np.stack([hv[c * NPC:(c + 1) * NPC] for c in range(NCORES)])
    h_own = np.zeros((NCORES, NPC, 128), np.float32)
    for c in range(NCORES):
        hi_ = min((c + 1) * NPC, N)
        if hi_ > c * NPC:
            h_own[c, :hi_ - c * NPC] = h[c * NPC:hi_]

    # ---- consts ----
    freqs = np.exp(-np.log(10000.0) * np.arange(64, dtype=np.float64) / 64)
    freq2 = np.concatenate([freqs, freqs]).astype(np.float32)[None, :]  # [1,128]
    pht = np.concatenate([np.zeros(64), np.full(64, 0.25)]
                         ).astype(np.float32)[:, None]  # phase in turns [128,1]
    iota_row = np.tile(np.arange(128, dtype=np.float16)[None, :], (128, 1))
    iota_col = np.arange(128, dtype=np.float32)[:, None]
    ones1 = np.ones((1, 128), np.float16)
    onesc = np.ones((128, 1), np.float16)

    # ================= build program =================
    nc = bacc.Bacc("TRN2", target_bir_lowering=False, debug=False,
                   num_devices=NCORES)

    def din(name, arr_one):  # declare input, shape of single-core array
        return nc.dram_tensor(name, list(arr_one.shape),
                              mybir.dt.from_np(arr_one.dtype),
                              kind="ExternalInput").ap()

    t_lo = din("hv_lo", hv_lo)
    t_hi = din("hv_hi", hv_hi)
    t_l = din("ltab", ltab)
    t_own = din("hv_own", hv_own[0])
    t_hf = din("h_own", h_own[0])
    t_ihv = din("idx_hv", idx_hv_p[0])
    t_il = din("idx_l", idx_l_p[0])
    t_iown = din("idx_own", idx_own)
    t_sl = din("sl_row", sl_row[0])
    t_slc = din("sl_col", sl_col[0])
    t_pos = nc.dram_tensor("posr", list(posr[0].shape), mybir.dt.float32r,
                           kind="ExternalInput").ap()
    wnames = dict(
        W_hj=W_hj.astype(np.float16), Wv3=Wv3.astype(np.float16),
        negWv3=(-Wv3).astype(np.float16), Wlb=Wlb.astype(np.float16),
        W_pd=W_pd.astype(np.float16), W_hi=W_hi.astype(np.float16),
        e_w2=e_w2.astype(np.float16),
        n_w1a=n_w1[0:128].astype(np.float16),
        n_w1b=n_w1[128:256].astype(np.float16),
        n_w2=n_w2.astype(np.float16),
        pht=pht, iota_row=iota_row, iota_col=iota_col,
        ones1=ones1, onesc=onesc,
        b2c=b2[:, None].astype(np.float32), b3c=b3[:, None].astype(np.float32),
        b4c=b4[:, None].astype(np.float32),

    )
    wt = {k: din(k, a) for k, a in wnames.items()}
    wt["freq2"] = nc.dram_tensor("freq2", [1, 128], mybir.dt.float32r,
                                 kind="ExternalInput").ap()
    wnames2 = dict(wnames); wnames2["freq2"] = freq2
    t_out = nc.dram_tensor("out", [NPC, 128], f32, kind="ExternalOutput").ap()

    with tile.TileContext(nc) as tc:
        import contextlib
        with contextlib.ExitStack() as ctx:
            cpool = ctx.enter_context(tc.tile_pool(name="consts", bufs=1))
            bpool = ctx.enter_context(tc.tile_pool(name="blk", bufs=2))
            kpool = ctx.enter_context(tc.tile_pool(name="chk", bufs=3))
            tpool = ctx.enter_context(tc.tile_pool(name="tl", bufs=4))
            p1 = ctx.enter_context(tc.tile_pool(name="p1", bufs=2, space="PSUM"))
            pk = ctx.enter_context(tc.tile_pool(name="pk", bufs=2, space="PSUM"))
            ps = ctx.enter_context(tc.tile_pool(name="ps", bufs=2, space="PSUM"))
            pt = ctx.enter_context(tc.tile_pool(name="pt", bufs=2, space="PSUM"))

            W = {}
            for k, a in wnames2.items():
                dt_ = (mybir.dt.float32r if k == "freq2"
                       else mybir.dt.from_np(a.dtype))
                tl_ = cpool.tile(list(a.shape), dt_, tag=k)
                nc.sync.dma_start(out=tl_[:], in_=wt[k][:])
                W[k] = tl_
            ident = cpool.tile([128, 128], f16, tag="ident")
            nc.vector.tensor_scalar(out=ident[:], in0=W["iota_row"][:],
                                    scalar1=W["iota_col"][:], scalar2=None,
                                    op0=OP.is_equal)

            for b in range(NBLK):
                capL, capH = int(caps[b, 0]), int(caps[b, 1])
                capB = capL + capH
                boff0 = int(caps[:b].sum())
                # --- own-node gather + t_b ---
                iown_b = bpool.tile([128, 8], i16, tag="iown_b")
                nc.sync.dma_start(out=iown_b[:],
                                  in_=t_iown[:, b * 8:(b + 1) * 8])
                g_own = bpool.tile([128, 2, 128], f16, tag="g_own")
                nc.gpsimd.dma_gather(g_own[:], t_own[:], iown_b[:],
                                     128, 128, 256, transpose=True)
                ptb = pt.tile([128, 128], f32, tag="ptmp")
                nc.tensor.matmul(out=ptb[:], lhsT=g_own[:, 0, :], rhs=W["W_hi"][:],
                                 start=True, stop=False)
                nc.tensor.matmul(out=ptb[:], lhsT=g_own[0:3, 1, :],
                                 rhs=W["negWv3"][:], start=False, stop=True)
                t_b = bpool.tile([128, 128], f16, tag="t_b")
                nc.scalar.activation(t_b[:], ptb[:], AF.Copy)

                # --- block loads ---
                sl_b = bpool.tile([1, capB], f16, tag="sl_b")
                nc.sync.dma_start(out=sl_b[:], in_=t_sl[0:1, boff0:boff0 + capB])
                pos_b = bpool.tile([1, capB], mybir.dt.float32r, tag="pos_b")
                nc.sync.dma_start(out=pos_b[:], in_=t_pos[0:1, boff0:boff0 + capB])
                slc_b = bpool.tile([128, capB // 128], f32, tag="slc_b")
                nc.sync.dma_start(out=slc_b[:],
                                  in_=t_slc[:, boff0 // 128:(boff0 + capB) // 128])
                ihv_b = bpool.tile([128, capB // 16], i16, tag="ihv_b")
                nc.sync.dma_start(out=ihv_b[:],
                                  in_=t_ihv[:, boff0 // 16:(boff0 + capB) // 16])
                il_b = bpool.tile([128, capB // 16], i16, tag="il_b")
                nc.sync.dma_start(out=il_b[:],
                                  in_=t_il[:, boff0 // 16:(boff0 + capB) // 16])
                hf_b = bpool.tile([128, 128], f32, tag="hf_b")
                nc.sync.dma_start(out=hf_b[:], in_=t_hf[b * 128:(b + 1) * 128, :])

                sums = ps.tile([128, 129], f32, tag="sums")
                first_sc = True
                boff = 0
                ntiles_blk = capB // 128
                tb_i = 0
                for sgi, cap in ((0, capL), (1, capH)):
                    tbl = t_lo if sgi == 0 else t_hi
                    done = 0
                    while done < cap:
                        Cc = min(CHUNK, cap - done)
                        o = boff + done
                        g_dst = kpool.tile([128, 2, Cc], f16, tag="g_dst")
                        nc.gpsimd.dma_gather(
                            g_dst[:], tbl[:],
                            ihv_b[:, o // 16:(o + Cc) // 16], Cc, Cc, 256,
                            transpose=True)
                        g_l = kpool.tile([128, 1, Cc], f16, tag="g_l")
                        nc.gpsimd.dma_gather(
                            g_l[:], t_l[:],
                            il_b[:, o // 16:(o + Cc) // 16], Cc, Cc, 128,
                            transpose=True)
                        pang = pk.tile([128, CHUNK], f32, tag="ktmp")
                        nc.tensor.matmul(
                            out=pang[:, :Cc],
                            lhsT=W["freq2"][:],
                            rhs=pos_b[0:1, o:o + Cc],
                            start=True, stop=True)
                        q_t = kpool.tile([128, CHUNK], f32, tag="q_t")
                        nc.vector.tensor_scalar(
                            out=q_t[:, :Cc], in0=pang[:, :Cc],
                            scalar1=1.0 / (2.0 * math.pi), scalar2=W["pht"][:],
                            op0=OP.mult, op1=OP.add)
                        qi_t = kpool.tile([128, CHUNK], mybir.dt.int32,
                                          tag="qi_t")
                        nc.vector.tensor_copy(qi_t[:, :Cc], q_t[:, :Cc])
                        qf_t = kpool.tile([128, CHUNK], f32, tag="qf_t")
                        nc.vector.tensor_copy(qf_t[:, :Cc], qi_t[:, :Cc])
                        d_t = kpool.tile([128, CHUNK], f32, tag="d_t")
                        nc.vector.tensor_tensor(out=d_t[:, :Cc],
                                                in0=q_t[:, :Cc],
                                                in1=qf_t[:, :Cc],
                                                op=OP.subtract)
                        pdT = kpool.tile([128, CHUNK], f16, tag="pdT")
                        nc.scalar.activation(pdT[:, :Cc], d_t[:, :Cc], AF.Sin,
                                             scale=2.0 * math.pi)
                        psl = pk.tile([128, CHUNK], f32, tag="ktmp")
                        nc.tensor.matmul(out=psl[:, :Cc], lhsT=W["ones1"][:],
                                         rhs=sl_b[0:1, o:o + Cc],
                                         start=True, stop=True)
                        indT = kpool.tile([128, CHUNK], f16, tag="indT")
                        nc.vector.tensor_scalar(out=indT[:, :Cc], in0=psl[:, :Cc],
                                                scalar1=W["iota_col"][:],
                                                scalar2=None, op0=OP.is_equal)
                        ps1 = p1.tile([128, CHUNK], f32, tag="ps1")
                        nc.tensor.matmul(out=ps1[:, :Cc], lhsT=W["W_hj"][:],
                                         rhs=g_dst[:, 0, :Cc], start=True,
                                         stop=False)
                        nc.tensor.matmul(out=ps1[:, :Cc], lhsT=W["Wv3"][:],
                                         rhs=g_dst[0:3, 1, :Cc], start=False,
                                         stop=False)
                        nc.tensor.matmul(out=ps1[:, :Cc], lhsT=W["Wlb"][:],
                                         rhs=g_l[0:7, 0, :Cc], start=False,
                                         stop=False)
                        nc.tensor.matmul(out=ps1[:, :Cc], lhsT=W["W_pd"][:],
                                         rhs=pdT[:, :Cc], start=False, stop=False)
                        nc.tensor.matmul(out=ps1[:, :Cc], lhsT=t_b[:],
                                         rhs=indT[:, :Cc], start=False, stop=True)
                        ef1 = kpool.tile([128, CHUNK], f16, tag="ef1")
                        nc.scalar.activation(ef1[:, :Cc], ps1[:, :Cc], AF.Silu)
                        for t in range(Cc // 128):
                            pe2 = pt.tile([128, 128], f32, tag="ptmp")
                            nc.tensor.matmul(out=pe2[:],
                                             lhsT=ef1[:, t * 128:(t + 1) * 128],
                                             rhs=W["e_w2"][:], start=True,
                                             stop=True)
                            ef2 = tpool.tile([128, 129], f16, tag="ef2")
                            nc.scalar.activation(ef2[:, 0:128], pe2[:], AF.Silu,
                                                 bias=W["b2c"][:])
                            nc.vector.tensor_copy(ef2[:, 128:129], W["onesc"][:])
                            ind = tpool.tile([128, 128], f16, tag="ind")
                            nc.vector.tensor_scalar(
                                out=ind[:], in0=W["iota_row"][:],
                                scalar1=slc_b[:, tb_i:tb_i + 1], scalar2=None,
                                op0=OP.is_equal)
                            nc.tensor.matmul(out=sums[:], lhsT=ind[:],
                                             rhs=ef2[:], start=first_sc,
                                             stop=(tb_i == ntiles_blk - 1))
                            first_sc = False
                            tb_i += 1
                        done += Cc
                    boff += cap

                # --- node MLP for this block ---
                inv = bpool.tile([128, 1], f32, tag="inv")
                nc.vector.tensor_scalar(out=inv[:], in0=sums[:, 128:129],
                                        scalar1=1.0, scalar2=None, op0=OP.max)
                inv2 = bpool.tile([128, 1], f32, tag="inv2")
                nc.vector.reciprocal(inv2[:], inv[:])
                agg = bpool.tile([128, 128], f16, tag="agg")
                nc.vector.tensor_scalar(out=agg[:], in0=sums[:, 0:128],
                                        scalar1=inv2[:], scalar2=None,
                                        op0=OP.mult)
                pat = pt.tile([128, 128], f16, tag="ptmp")
                nc.tensor.transpose(out=pat[:], in_=agg[:], identity=ident[:])
                aggT = bpool.tile([128, 128], f16, tag="aggT")
                nc.scalar.activation(aggT[:], pat[:], AF.Copy)
                p3 = pt.tile([128, 128], f32, tag="ptmp")
                nc.tensor.matmul(out=p3[:], lhsT=W["n_w1a"][:],
                                 rhs=g_own[:, 0, :], start=True, stop=False)
                nc.tensor.matmul(out=p3[:], lhsT=W["n_w1b"][:], rhs=aggT[:],
                                 start=False, stop=True)
                o1 = bpool.tile([128, 128], f16, tag="o1")
                nc.scalar.activation(o1[:], p3[:], AF.Silu, bias=W["b3c"][:])
                p4 = pt.tile([128, 128], f32, tag="ptmp")
                nc.tensor.matmul(out=p4[:], lhsT=W["n_w2"][:], rhs=o1[:],
                                 start=True, stop=True)
                o2 = bpool.tile([128, 128], f16, tag="o2")
                nc.scalar.activation(o2[:], p4[:], AF.Silu, bias=W["b4c"][:])
                po = pt.tile([128, 128], f16, tag="ptmp")
                nc.tensor.transpose(out=po[:], in_=o2[:], identity=ident[:])
                ob = bpool.tile([128, 128], f32, tag="ob")
                nc.vector.tensor_tensor(out=ob[:], in0=po[:], in1=hf_b[:],
                                        op=OP.add)
                nc.sync.dma_start(out=t_out[b * 128:(b + 1) * 128, :], in_=ob[:])

    nc.compile()

    in_maps = []
    for c in range(NCORES):
        m = dict(hv_lo=hv_lo, hv_hi=hv_hi, ltab=ltab, hv_own=hv_own[c],
                 h_own=h_own[c], idx_hv=idx_hv_p[c], idx_l=idx_l_p[c],
                 idx_own=idx_own, sl_row=sl_row[c], sl_col=sl_col[c],
                 posr=posr[c])
        m.update(wnames)
        m["freq2"] = freq2
        in_maps.append(m)
    import os
    kr = run_bass_kernel_spmd(nc, in_maps, list(range(NCORES)),
                              trace=bool(os.environ.get("KTRACE")))
    global LAST_RESULTS, LAST_NC, LAST_INMAPS
    LAST_RESULTS = kr
    LAST_NC = nc
    LAST_INMAPS = in_maps
    res = kr.results
    out = np.concatenate([res[c]["out"] for c in range(NCORES)], 0)[:N]
    return out.astype(np.float32)

